# revision 1
# baseline (speedup 1.0000x reference)
"""LIF v6: memory-roofline DMA schedule + HW-valid engine rebalance.

Recurrence (states u_t = pre-threshold potential, W_t = 0.9*w_t adaptation):
    s_t     = 1[u_t > 0.5]                      (spike, output via ScalarE Sign)
    W_{t+1} = 0.9*W_t + 0.045*(u_t + s_t)       split:
               [0:FD)  custom DVE LIF_W (one fused op)
               [FD:F)  GpSimd TS/TT chain using the int8 Sign output:
                         y = 0.0225*sgn + 0.0225   (= 0.045*s_t)
                         k = 0.045*u ; q = 0.9*W
                         W' = q + (k + y)
    vp_{t+1} = x_{t+1} - W_t                    GpSimd TT subtract
    u_{t+1} = 0.45*u_t - 0.3*s_t + vp_{t+1}     custom DVE LIF_U
vp is computed one step ahead (it only needs W) so the DVE u-chain never
waits on a same-step Pool->DVE semaphore.

Schedule: per-core traffic (26.2MB x in + 6.5MB s out) at ~332GB/s is a
~99us DMA floor — the kernel is memory-bound.  x loads are enqueued
up-front on the SP queue with geometrically ramped chunk sizes (transfer
is faster per step than compute, so compute never stalls after the ramp);
s stores queue behind all loads on SP so they never pace compute.

Output via ScalarE: sgn_t = Sign(u_t - 0.5) -> int8 in {-1,0,1}; host maps
(sgn > 0) -> {0,1} f32.  Derivation: u = 0.5*mem + x - w with
mem' = u - 0.5 s, w' = 0.9 w + 0.05 (u + s), W := 0.9 w.
"""

import numpy as np

import concourse.bass as bass
import concourse.bacc as bacc
import concourse.mybir as mybir
import concourse.tile as tile
from concourse.bass_utils import run_bass_kernel_spmd

import concourse.dve_ops as dops
from concourse.dve_ops import DveOp
from concourse.dve_spec import Spec, Src0, Src1, C0, C1, C2, lower
from concourse.dve_ops import has_src1
from concourse.dve_uop import DveOpSpec

B, N, T = 64, 8192, 100
N_CORES = 8
P = 128
FD = 400  # free-dim split for the W update: [0:FD) on DVE, [FD:F) on GpSimd
# U-update split disabled (UG >= F): routing part of the u-chain through
# Pool puts the Act Sign on the serial u recurrence and costs ~12us.
UG = 1 << 30

F32 = mybir.dt.float32
I8 = mybir.dt.int8
Alu = mybir.AluOpType
Act = mybir.ActivationFunctionType


def _register(name, spec):
    for o in dops.OPS:
        if o.name == name:
            return o
    opcode = dops._CUSTOM_DVE_ROW_BASE + len(dops.OPS)
    assert opcode < 0x20
    shas = {}
    for ver in ("v3", "v4"):
        dspec = DveOpSpec(
            name=name, opcode=opcode, uops=lower(spec, ver=ver),
            rd1_en=has_src1(spec),
        )
        shas[ver] = dspec.sha(ver)
    op = DveOp(name, spec, subdim=False, uops_sha=shas)
    dops.OPS.append(op)
    dops._SUB_OPCODE_FOR_NAME[name] = opcode
    dops.CUSTOM_DVE_SPECS[name] = spec
    return op


# w' = s0*in1 + s1*(in0 + (in0 > imm2))
LIF_W = _register(
    "LIF_W_ANT",
    Spec(
        body=Src1 * C0 + (Src0 + (Src0 > C2)) * C1,
        reference=lambda in0, in1, s0, s1, imm2: in1 * s0
        + (in0 + (in0 > imm2).astype(np.float32)) * s1,
    ),
)

# u' = s0*in0 - s1*(in0 > imm2) + in1
LIF_U = _register(
    "LIF_U_ANT",
    Spec(
        body=Src0 * C0 - (Src0 > C2) * C1 + Src1,
        reference=lambda in0, in1, s0, s1, imm2: in0 * s0
        - (in0 > imm2).astype(np.float32) * s1
        + in1,
    ),
)


def _x_plan(T_: int) -> list[tuple[int, int]]:
    """Chunk plan (t0, n): geometric ramp.  DMA transfers ~0.8us per step of
    x while compute consumes ~1.0us/step, so chunk i+1 (loading while chunk i
    is consumed) may be ~1.25x the steps already loaded without ever stalling
    compute.  A small trailing chunk keeps the post-load compute tail short."""
    sizes = []
    t = 0
    n = 2
    while t < T_:
        n = min(n, T_ - t)
        sizes.append(n)
        t += n
        n = min(20, max(n + 1, int(0.25 * t)))
    # shrink the tail: split a large trailing chunk so the last load is small
    if len(sizes) >= 2 and sizes[-1] >= 12:
        last = sizes.pop()
        sizes.extend([last - 8, 8])
    plan = []
    t = 0
    for n in sizes:
        plan.append((t, n))
        t += n
    return plan


_NC_CACHE: dict = {}


def build_nc(T_: int, P_: int, F_: int, sch: int = 10, fd: int | None = None,
             ug: int | None = None):
    key = (T_, P_, F_, sch, fd, ug)
    if key in _NC_CACHE:
        return _NC_CACHE[key]
    nc = _build_nc(T_, P_, F_, sch, fd, ug)
    _NC_CACHE[key] = nc
    return nc


def _build_nc(T_: int, P_: int, F_: int, sch: int = 10, fd: int | None = None,
              ug: int | None = None):
    if fd is None:
        fd = min(FD, F_)
    fd = min(fd, F_)
    if ug is None:
        ug = min(UG, F_)
    ug = min(ug, F_)
    mn = min(fd, ug)
    nc = bacc.Bacc("TRN2", target_bir_lowering=False, debug=False)
    E = P_ * F_
    x_d = nc.dram_tensor("x", [T_, E], F32, kind="ExternalInput").ap()
    s_d = nc.dram_tensor("s", [T_, E], I8, kind="ExternalOutput").ap()

    plan = _x_plan(T_)
    xslot = max(n for _, n in plan)
    t2chunk = {}
    for i, (t0, n) in enumerate(plan):
        for tt in range(t0, t0 + n):
            t2chunk[tt] = (i, tt - t0)

    # store plan: uniform sch-step chunks, small trailing chunks so the
    # final store (which must wait for the last Sign) is short
    s_plan = []
    t = 0
    while t < T_:
        n = min(sch, T_ - t)
        if T_ - t <= sch and n > 6:
            s_plan.append((t, n - 4))
            s_plan.append((t + n - 4, 4))
            t = T_
        else:
            s_plan.append((t, n))
            t += n
    n_sch = len(s_plan)
    t2sch = {}
    for i, (t0, n) in enumerate(s_plan):
        for tt in range(t0, t0 + n):
            t2sch[tt] = (i, tt - t0)

    with tile.TileContext(nc) as tc:
        with (
            tc.tile_pool(name="xp", bufs=3) as xp,
            # all store chunks stay resident: stores are deferred to the tail
            # of the SP queue (behind every load) so they never pace compute
            tc.tile_pool(name="sp", bufs=n_sch) as sp,
            tc.tile_pool(name="st", bufs=3) as st,
            tc.tile_pool(name="zp", bufs=1) as zp,
        ):
            x_tiles = []

            def load_chunk(i):
                t0, n_t = plan[i]
                xt = xp.tile([P_, xslot * F_], F32, tag="x")
                dst = xt[:].rearrange("p (t f) -> p t f", t=xslot)
                src = x_d[t0:t0 + n_t].rearrange("t (p f) -> p t f", p=P_)
                nc.sync.dma_start(dst[:, :n_t], src)
                x_tiles.append(xt)

            def x_slice(t):
                i, off = t2chunk[t]
                return x_tiles[i][:, off * F_:(off + 1) * F_]

            # Enqueue every x load up-front: the SP queue streams them
            # back-to-back, paced only by tile-slot reuse.
            for i in range(len(plan)):
                load_chunk(i)

            s_chunk = sp.tile([P_, sch * F_], I8, tag="s")
            w_zero = zp.tile([P_, F_], F32, tag="wz")
            nc.gpsimd.memset(w_zero[:], 0.0)
            bias_m05 = zp.tile([P_, 1], F32, tag="b05")
            nc.gpsimd.memset(bias_m05[:], -0.5)

            u_prev = None
            w_prev = w_zero
            vp_next = None  # vp for step t (consumed by U at step t)
            for t in range(T_):
                sk, stl = t2sch[t]
                u = u_prev if t > 0 else x_slice(0)

                # spike output: Sign(u - 0.5) -> int8 {-1,0,1}.  The [fd:)
                # slice is emitted first: the Pool W-update chain consumes it,
                # so it should not wait behind the full-width Sign.
                sg = s_chunk[:, stl * F_:(stl + 1) * F_]
                if mn < F_ and t + 1 < T_:
                    nc.scalar.activation(
                        sg[:, mn:], u[:, mn:], Act.Sign, bias=bias_m05[:], scale=1.0
                    )
                    nc.scalar.activation(
                        sg[:, :mn], u[:, :mn], Act.Sign, bias=bias_m05[:], scale=1.0
                    )
                else:
                    nc.scalar.activation(
                        sg[:], u[:], Act.Sign, bias=bias_m05[:], scale=1.0
                    )

                if t + 1 < T_:
                    # W_{t+1} = 0.9*W_t + 0.045*(u_t + s_t); split DVE/Pool
                    v_new = st.tile([P_, F_], F32, tag="v")
                    if fd > 0:
                        nc.vector._custom_dve(
                            LIF_W,
                            out=v_new[:, :fd], in0=u[:, :fd], in1=w_prev[:, :fd],
                            s0=0.9, s1=0.045, imm2=0.5,
                        )
                    if fd < F_:
                        fr = F_ - fd
                        q = st.tile([P_, fr], F32, tag="q")
                        nc.gpsimd.tensor_scalar(
                            q[:], w_prev[:, fd:], 0.9, None, op0=Alu.mult
                        )
                        k = st.tile([P_, fr], F32, tag="k")
                        nc.gpsimd.tensor_scalar(
                            k[:], u[:, fd:], 0.045, None, op0=Alu.mult
                        )
                        y = st.tile([P_, fr], F32, tag="y")
                        nc.gpsimd.tensor_scalar(
                            y[:], sg[:, fd:], 0.0225, 0.0225,
                            op0=Alu.mult, op1=Alu.add,
                        )
                        m = st.tile([P_, fr], F32, tag="m")
                        nc.gpsimd.tensor_tensor(m[:], k[:], y[:], op=Alu.add)
                        nc.gpsimd.tensor_tensor(
                            v_new[:, fd:], q[:], m[:], op=Alu.add
                        )

                    vp = vp_next if t > 0 else x_slice(1)
                    u_new = st.tile([P_, F_], F32, tag="u")
                    if ug > 0:
                        nc.vector._custom_dve(
                            LIF_U, out=u_new[:, :ug], in0=u[:, :ug],
                            in1=vp[:, :ug], s0=0.45, s1=0.3, imm2=0.5,
                        )
                    if ug < F_:
                        # u' = 0.45*u + vp - 0.3*s, with -0.3*s from the int8
                        # sign: g2 = -0.15*sgn - 0.15
                        ur = F_ - ug
                        g2 = st.tile([P_, ur], F32, tag="g2")
                        nc.gpsimd.tensor_scalar(
                            g2[:], sg[:, ug:], -0.15, -0.15,
                            op0=Alu.mult, op1=Alu.add,
                        )
                        h = st.tile([P_, ur], F32, tag="h")
                        nc.gpsimd.tensor_tensor(
                            h[:], g2[:], vp[:, ug:], op=Alu.add
                        )
                        kk = st.tile([P_, ur], F32, tag="kk")
                        nc.gpsimd.tensor_scalar(
                            kk[:], u[:, ug:], 0.45, None, op0=Alu.mult
                        )
                        nc.gpsimd.tensor_tensor(
                            u_new[:, ug:], h[:], kk[:], op=Alu.add
                        )

                    # lookahead: vp_{t+2} = x_{t+2} - W_{t+1} (just produced)
                    if t + 2 < T_:
                        vp_next = st.tile([P_, F_], F32, tag="vp")
                        nc.gpsimd.tensor_tensor(
                            vp_next[:], x_slice(t + 2)[:], v_new[:],
                            op=Alu.subtract,
                        )
                    u_prev, w_prev = u_new, v_new

                st0, sn = s_plan[sk]
                if stl == sn - 1:
                    dst = s_d[st0:st0 + sn].rearrange("t (p f) -> p t f", p=P_)
                    # SP queue, behind all loads: transfers drain there
                    # once the load stream finishes, never pacing compute.
                    nc.sync.dma_start(
                        dst,
                        s_chunk[:].rearrange("p (t f) -> p t f", t=sch)[:, :sn],
                    )
                    if t + 1 < T_:
                        s_chunk = sp.tile([P_, sch * F_], I8, tag="s")
    nc.compile()
    return nc


def postprocess_core(core_result: dict) -> np.ndarray:
    return (core_result["s"].T > 0).astype(np.float32)


def _run(x: np.ndarray, trace: bool = False):
    x = np.asarray(x)
    b, n, t_ = x.shape
    e_tot = b * n
    e = e_tot // N_CORES
    f = e // P
    nc = build_nc(t_, P, f)
    xf = x.reshape(e_tot, t_)
    in_maps = [
        {"x": np.ascontiguousarray(xf[c * e:(c + 1) * e].T)}
        for c in range(N_CORES)
    ]
    bkr = run_bass_kernel_spmd(nc, in_maps, list(range(N_CORES)), trace=False)
    res = bkr.results
    out = np.concatenate([postprocess_core(res[c]) for c in range(N_CORES)], axis=0)
    return np.ascontiguousarray(out.reshape(b, n, t_)).astype(np.float32), bkr


def kernel(x: np.ndarray) -> np.ndarray:
    return _run(x)[0]



# revision 2
# speedup vs baseline: 406.8821x; 406.8821x over previous
"""LIF v7: PE-offloaded vp + DVE recurrence, engine-balanced schedule.

Recurrence (u_t = pre-threshold potential, W_t = 0.9*w_t adaptation):
    s_t     = 1[u_t > 0.5]                      ScalarE Sign -> int8
    W_{t+1} = 0.9*W_t + 0.045*(u_t + s_t)       custom DVE LIF_W
    vp_t    = x_{t+1} - W_t                     TensorE: two scaled-identity
                                                matmuls accumulated in PSUM
    u_{t+1} = 0.45*u_t - 0.3*s_t + vp_t         custom DVE LIF_U (reads PSUM)

Engine budget per step (measured): DVE 2x727ns (LIF_W; LIF_U), ScalarE
612ns (Sign), PE 2x429ns (fp32 identity passes, off critical path via
1-step lookahead), GpSimd idle.  DVE is the pacer at ~1.45us/step.

Schedule: x loads enqueued up-front on the SP queue with geometrically
ramped chunk sizes; s stores queue behind all loads so they never pace
compute.  Host maps (sgn > 0) -> {0,1} f32.
"""

import numpy as np

import concourse.bass as bass
import concourse.bacc as bacc
import concourse.mybir as mybir
import concourse.tile as tile
from concourse.bass_utils import run_bass_kernel_spmd
from concourse.masks import make_identity

import concourse.dve_ops as dops
from concourse.dve_ops import DveOp
from concourse.dve_spec import Spec, Src0, Src1, C0, C1, C2, lower
from concourse.dve_ops import has_src1
from concourse.dve_uop import DveOpSpec

B, N, T = 64, 8192, 100
N_CORES = 8
P = 128

F32 = mybir.dt.float32
I8 = mybir.dt.int8
Alu = mybir.AluOpType
Act = mybir.ActivationFunctionType


def _register(name, spec):
    for o in dops.OPS:
        if o.name == name:
            return o
    opcode = dops._CUSTOM_DVE_ROW_BASE + len(dops.OPS)
    assert opcode < 0x20
    shas = {}
    for ver in ("v3", "v4"):
        dspec = DveOpSpec(
            name=name, opcode=opcode, uops=lower(spec, ver=ver),
            rd1_en=has_src1(spec),
        )
        shas[ver] = dspec.sha(ver)
    op = DveOp(name, spec, subdim=False, uops_sha=shas)
    dops.OPS.append(op)
    dops._SUB_OPCODE_FOR_NAME[name] = opcode
    dops.CUSTOM_DVE_SPECS[name] = spec
    return op


# w' = s0*in1 + s1*(in0 + (in0 > imm2))
LIF_W = _register(
    "LIF_W_ANT",
    Spec(
        body=Src1 * C0 + (Src0 + (Src0 > C2)) * C1,
        reference=lambda in0, in1, s0, s1, imm2: in1 * s0
        + (in0 + (in0 > imm2).astype(np.float32)) * s1,
    ),
)

# u' = s0*in0 - s1*(in0 > imm2) + in1
LIF_U = _register(
    "LIF_U_ANT",
    Spec(
        body=Src0 * C0 - (Src0 > C2) * C1 + Src1,
        reference=lambda in0, in1, s0, s1, imm2: in0 * s0
        - (in0 > imm2).astype(np.float32) * s1
        + in1,
    ),
)


def _x_plan(T_: int) -> list[tuple[int, int]]:
    """Chunk plan (t0, n): geometric ramp so transfers stay ahead of
    compute without a large first-chunk stall."""
    sizes = []
    t = 0
    n = 3
    while t < T_:
        n = min(n, T_ - t)
        sizes.append(n)
        t += n
        n = min(20, max(n + 1, int(0.25 * t)))
    if len(sizes) >= 2 and sizes[-1] >= 12:
        last = sizes.pop()
        sizes.extend([last - 8, 8])
    plan = []
    t = 0
    for n in sizes:
        plan.append((t, n))
        t += n
    return plan


_NC_CACHE: dict = {}


def build_nc(T_: int, P_: int, F_: int, sch: int = 10):
    key = (T_, P_, F_, sch)
    if key in _NC_CACHE:
        return _NC_CACHE[key]
    nc = _build_nc(T_, P_, F_, sch)
    _NC_CACHE[key] = nc
    return nc


def _build_nc(T_: int, P_: int, F_: int, sch: int = 10):
    nc = bacc.Bacc("TRN2", target_bir_lowering=False, debug=False)
    E = P_ * F_
    x_d = nc.dram_tensor("x", [T_, E], F32, kind="ExternalInput").ap()
    s_d = nc.dram_tensor("s", [T_, E], I8, kind="ExternalOutput").ap()

    plan = _x_plan(T_)
    xslot = max(n for _, n in plan)
    t2chunk = {}
    for i, (t0, n) in enumerate(plan):
        for tt in range(t0, t0 + n):
            t2chunk[tt] = (i, tt - t0)

    # store plan: uniform sch-step chunks, small trailing chunk so the
    # final store (waiting on the last Sign) is short
    s_plan = []
    t = 0
    while t < T_:
        n = min(sch, T_ - t)
        if T_ - t <= sch and n > 6:
            s_plan.append((t, n - 4))
            s_plan.append((t + n - 4, 4))
            t = T_
        else:
            s_plan.append((t, n))
            t += n
    n_sch = len(s_plan)
    t2sch = {}
    for i, (t0, n) in enumerate(s_plan):
        for tt in range(t0, t0 + n):
            t2sch[tt] = (i, tt - t0)

    with tile.TileContext(nc) as tc:
        with (
            tc.tile_pool(name="xp", bufs=3) as xp,
            # all store chunks stay resident; stores queue behind loads
            tc.tile_pool(name="sp", bufs=n_sch) as sp,
            tc.tile_pool(name="up", bufs=2) as up,
            tc.tile_pool(name="wp", bufs=2) as wp,
            tc.tile_pool(name="zp", bufs=1) as zp,
            tc.tile_pool(name="vp", bufs=2, space="PSUM") as vpool,
        ):
            x_tiles = []

            def load_chunk(i):
                t0, n_t = plan[i]
                xt = xp.tile([P_, xslot * F_], F32, tag="x")
                dst = xt[:].rearrange("p (t f) -> p t f", t=xslot)
                src = x_d[t0:t0 + n_t].rearrange("t (p f) -> p t f", p=P_)
                nc.sync.dma_start(dst[:, :n_t], src)
                x_tiles.append(xt)

            def x_slice(t):
                i, off = t2chunk[t]
                return x_tiles[i][:, off * F_:(off + 1) * F_]

            for i in range(len(plan)):
                load_chunk(i)

            s_chunk = sp.tile([P_, sch * F_], I8, tag="s")
            w_zero = zp.tile([P_, F_], F32, tag="wz")
            nc.gpsimd.memset(w_zero[:], 0.0)
            bias_m05 = zp.tile([P_, 1], F32, tag="b05")
            nc.gpsimd.memset(bias_m05[:], -0.5)
            ident_p = zp.tile([P_, P_], F32, tag="idp")
            ident_n = zp.tile([P_, P_], F32, tag="idn")
            make_identity(nc, ident_p[:])
            nc.vector.tensor_scalar(
                ident_n[:], ident_p[:], -1.0, None, op0=Alu.mult
            )

            u_prev = None
            w_prev = w_zero
            vp_cur = None  # vp_t, consumed by LIF_U at step t
            for t in range(T_):
                sk, stl = t2sch[t]
                u = u_prev if t > 0 else x_slice(0)

                sg = s_chunk[:, stl * F_:(stl + 1) * F_]
                nc.scalar.activation(
                    sg[:], u[:], Act.Sign, bias=bias_m05[:], scale=1.0
                )

                if t + 1 < T_:
                    # W_{t+1} = 0.9*W_t + 0.045*(u_t + s_t)   [DVE]
                    w_new = wp.tile([P_, F_], F32, tag="w")
                    nc.vector._custom_dve(
                        LIF_W,
                        out=w_new[:], in0=u[:], in1=w_prev[:],
                        s0=0.9, s1=0.045, imm2=0.5,
                    )

                    # vp_{t+1} = x_{t+2} - W_{t+1}   [PE, into PSUM]
                    if t + 2 < T_:
                        vp_next = vpool.tile([P_, F_], F32, tag="vp")
                        nc.tensor.matmul(
                            vp_next[:], ident_p[:], x_slice(t + 2)[:],
                            start=True, stop=False,
                        )
                        nc.tensor.matmul(
                            vp_next[:], ident_n[:], w_new[:],
                            start=False, stop=True,
                        )
                    else:
                        vp_next = None

                    # u_{t+1} = 0.45*u_t - 0.3*s_t + vp_t   [DVE]
                    vp = vp_cur if t > 0 else x_slice(1)
                    u_new = up.tile([P_, F_], F32, tag="u")
                    nc.vector._custom_dve(
                        LIF_U, out=u_new[:], in0=u[:],
                        in1=vp[:], s0=0.45, s1=0.3, imm2=0.5,
                    )
                    u_prev, w_prev, vp_cur = u_new, w_new, vp_next

                st0, sn = s_plan[sk]
                if stl == sn - 1:
                    dst = s_d[st0:st0 + sn].rearrange("t (p f) -> p t f", p=P_)
                    nc.sync.dma_start(
                        dst,
                        s_chunk[:].rearrange("p (t f) -> p t f", t=sch)[:, :sn],
                    )
                    if t + 1 < T_:
                        s_chunk = sp.tile([P_, sch * F_], I8, tag="s")
    nc.compile()
    return nc


def postprocess_core(core_result: dict) -> np.ndarray:
    return (core_result["s"].T > 0).astype(np.float32)


def _run(x: np.ndarray):
    x = np.asarray(x)
    b, n, t_ = x.shape
    e_tot = b * n
    e = e_tot // N_CORES
    f = e // P
    nc = build_nc(t_, P, f)
    xf = x.reshape(e_tot, t_)
    in_maps = [
        {"x": np.ascontiguousarray(xf[c * e:(c + 1) * e].T)}
        for c in range(N_CORES)
    ]
    bkr = run_bass_kernel_spmd(nc, in_maps, list(range(N_CORES)), trace=False)
    res = bkr.results
    out = np.concatenate(
        [postprocess_core(res[c]) for c in range(N_CORES)], axis=0
    )
    return np.ascontiguousarray(out.reshape(b, n, t_)).astype(np.float32), bkr


def kernel(x: np.ndarray) -> np.ndarray:
    return _run(x)[0]
